# revision 19
# baseline (speedup 1.0000x reference)
"""Trainium2 Bass kernel for nn_CrossAttention_42365557408181.

Dual-query cross-attention: B=2, N=4096 (query rows), M=1024 (context rows),
H=8 heads, DH=64, QD=512, CD=768. Returns (out [B,N,512] f32,
probs_avg [B,N,1024] f32 = mean over heads of scaled raw logits).

Sharding: pure data-parallel over (B x N) -> 8 shards of 1024 query rows,
one per NeuronCore; k/v are recomputed per core for its batch (no
collectives). All device matmuls contract on the partition dim, so the host
pre-transposes activations (xT/aT/ctxT) and the attention is computed in a
"simT" [j, i] layout:

  qT = Wq^T-contract(xT)              [512(hd), 1024(i)]
  kT = (Wk*scale)^T-contract(ctxT)    [512(hd), 1024(j)]   (scale folded in)
  v  = ctxT^T-contract(Wv)            [1024(j), 512(hd)]
  per head/pass: simT = kT_h .T-contract. qT_h  -> exp (no max-sub; |sim|<~6)
  U = ET .T-contract. [v_h | ones]    [i, 65]  (col 64 = softmax denominator)
  out_h = U[:, :64] / U[:, 64]        summed over both query passes
  probsT = kT .T-contract. qT * (1/H) (single K=512 matmul = sum over heads)
  final  = merged_out @ Wo + bo       (PE-transpose of merged out feeds Wo)

Compute dtype bf16 (f32 PSUM accumulation), outputs f32.
"""

import os
import sys

for _p in ("/opt/trn_rl_repo", "/root/.axon_site/_ro/trn_rl_repo"):
    if os.path.isdir(_p) and _p not in sys.path:
        sys.path.insert(0, _p)

import numpy as np
import ml_dtypes

B, N, M = 2, 4096, 1024
QD, CD, H, DH = 512, 768, 8, 64
SCALE = DH ** -0.5
NCORES = 8
NSH = (B * N) // NCORES  # 1024 query rows per core

_cache = {}


def _build():
    import concourse.bass as bass
    import concourse.mybir as mybir
    import concourse.tile as tile

    f32 = mybir.dt.float32
    bf16 = mybir.dt.bfloat16
    AF = mybir.ActivationFunctionType

    nc = bass.Bass("TRN2")

    xT_d = nc.declare_dram_parameter("xT", [QD, NSH], bf16, isOutput=False)
    aT_d = nc.declare_dram_parameter("aT", [QD, NSH], bf16, isOutput=False)
    ctxT_d = nc.declare_dram_parameter("ctxT", [CD, M], bf16, isOutput=False)
    wq_d = nc.declare_dram_parameter("wq", [QD, QD], bf16, isOutput=False)
    wk_d = nc.declare_dram_parameter("wk", [CD, QD], bf16, isOutput=False)
    wv_d = nc.declare_dram_parameter("wv", [CD, QD], bf16, isOutput=False)
    wo_d = nc.declare_dram_parameter("wo", [QD, QD], bf16, isOutput=False)
    bob_d = nc.declare_dram_parameter("bob", [128, QD], f32, isOutput=False)
    out_d = nc.declare_dram_parameter("out", [NSH, QD], f32, isOutput=True)
    probsT_d = nc.declare_dram_parameter("probsT", [M, NSH], f32, isOutput=True)

    QT, CT = QD // 128, CD // 128          # 4, 6 k-tiles
    IT, JT = NSH // 128, M // 128          # 8, 8 row tiles
    IC = NSH // 512                        # 2 i-chunks of 512

    with tile.TileContext(nc) as tc:
        with (
            tc.tile_pool(name="w", bufs=1) as wpool,
            tc.tile_pool(name="act", bufs=1) as apool,
            tc.tile_pool(name="et", bufs=20) as etpool,
            tc.tile_pool(name="small", bufs=4) as spool,
            tc.tile_pool(name="stage", bufs=4) as stpool,
            tc.tile_pool(name="ut", bufs=4) as utpool,
            tc.tile_pool(name="psim", bufs=2, space="PSUM") as psim,
            tc.tile_pool(name="pmix", bufs=4, space="PSUM") as pmix,
        ):
            # ---- load inputs ----
            xT = apool.tile([128, QT, NSH], bf16, tag="xT")
            aT = apool.tile([128, QT, NSH], bf16, tag="aT")
            ctxT = apool.tile([128, CT, M], bf16, tag="ctxT")
            wq = wpool.tile([128, QT, QD], bf16, tag="wq")
            wk = wpool.tile([128, CT, QD], bf16, tag="wk")
            wv = wpool.tile([128, CT, QD], bf16, tag="wv")
            wo = wpool.tile([128, QT, QD], bf16, tag="wo")
            bob = wpool.tile([128, QD], f32, tag="bob")

            nc.sync.dma_start(ctxT[:], ctxT_d.ap().rearrange("(t p) j -> p t j", p=128))
            nc.sync.dma_start(wk[:], wk_d.ap().rearrange("(t p) d -> p t d", p=128))
            nc.sync.dma_start(wq[:], wq_d.ap().rearrange("(t p) d -> p t d", p=128))
            nc.sync.dma_start(xT[:], xT_d.ap().rearrange("(t p) i -> p t i", p=128))
            nc.sync.dma_start(wv[:], wv_d.ap().rearrange("(t p) d -> p t d", p=128))
            nc.sync.dma_start(aT[:], aT_d.ap().rearrange("(t p) i -> p t i", p=128))
            nc.sync.dma_start(wo[:], wo_d.ap().rearrange("(t p) d -> p t d", p=128))
            nc.sync.dma_start(bob[:], bob_d.ap())

            # ---- projections (emitted interleaved with the head loop so
            # ScalarE starts exp work ~7us in instead of after the whole
            # projection phase) ----
            qT = [apool.tile([128, NSH], bf16, tag=f"qT{dt}", name=f"qT{dt}") for dt in range(QT)]
            kT = [apool.tile([128, M], bf16, tag=f"kT{dt}", name=f"kT{dt}") for dt in range(QT)]
            vaug = [apool.tile([128, H * 65], bf16, tag=f"vaug{jt}", name=f"vaug{jt}") for jt in range(JT)]
            outT = [apool.tile([128, NSH], bf16, tag=f"outT{dt}", name=f"outT{dt}") for dt in range(QT)]

            def proj_dt(dt):
                for jc in range(M // 512):
                    ps = pmix.tile([128, 512], f32, tag="mix", name=f"psk{dt}{jc}")
                    for ct in range(CT):
                        nc.tensor.matmul(
                            ps[:],
                            wk[:, ct, dt * 128:(dt + 1) * 128],
                            ctxT[:, ct, jc * 512:(jc + 1) * 512],
                            start=(ct == 0), stop=(ct == CT - 1),
                        )
                    nc.vector.tensor_copy(kT[dt][:, jc * 512:(jc + 1) * 512], ps[:])
                for ic in range(IC):
                    ps = pmix.tile([128, 512], f32, tag="mix", name=f"psq{dt}{ic}")
                    for ct in range(QT):
                        nc.tensor.matmul(
                            ps[:],
                            wq[:, ct, dt * 128:(dt + 1) * 128],
                            xT[:, ct, ic * 512:(ic + 1) * 512],
                            start=(ct == 0), stop=(ct == QT - 1),
                        )
                    nc.vector.tensor_copy(qT[dt][:, ic * 512:(ic + 1) * 512], ps[:])

            def proj_v():
                for jt in range(JT):
                    nc.vector.memset(
                        vaug[jt].rearrange("p (h c) -> p h c", c=65)[:, :, 64:65], 1.0
                    )
                    ps = pmix.tile([128, 512], f32, tag="mix", name=f"psv{jt}")
                    for ct in range(CT):
                        nc.tensor.matmul(
                            ps[:],
                            ctxT[:, ct, jt * 128:(jt + 1) * 128],
                            wv[:, ct, :],
                            start=(ct == 0), stop=(ct == CT - 1),
                        )
                    nc.vector.tensor_copy(
                        vaug[jt].rearrange("p (h c) -> p h c", c=65)[:, :, 0:64],
                        ps[:].rearrange("p (h c) -> p h c", c=64),
                    )

            def sims_exps(h, qh):
                ets = []
                for jt in range(JT):
                    p0 = 64 * (h % 2)
                    kh = kT[h // 2][p0:p0 + 64, :]
                    ps = psim.tile([128, NSH], f32, tag="sim", name=f"sim{h}{jt}")
                    for ic in range(IC):
                        nc.tensor.matmul(
                            ps[:, ic * 512:(ic + 1) * 512],
                            kh[:, jt * 128:(jt + 1) * 128],
                            qh[:, ic * 512:(ic + 1) * 512],
                            start=True, stop=True,
                        )
                    et = etpool.tile([128, NSH], bf16, tag="et", name=f"et{h}{jt}")
                    nc.scalar.activation(et[:], ps[:], AF.Exp)
                    ets.append(et)
                return ets

            def ut_norm(h, is_x, ets):
                p0 = 64 * (h % 2)
                ut = utpool.tile([65, NSH], bf16, tag="ut", name=f"ut{h}{is_x}")
                for ic in range(IC):
                    psu = pmix.tile([65, 512], f32, tag="mix", name=f"psu{h}{ic}")
                    for jt in range(JT):
                        nc.tensor.matmul(
                            psu[:],
                            vaug[jt][:, h * 65:(h + 1) * 65],
                            ets[jt][:, ic * 512:(ic + 1) * 512],
                            start=(jt == 0), stop=(jt == JT - 1),
                        )
                    nc.vector.tensor_copy(ut[:, ic * 512:(ic + 1) * 512], psu[:])
                dsc = spool.tile([128, 8], bf16, tag="dsc", name=f"dsc{h}{is_x}")
                nc.sync.dma_start(
                    dsc[:], ut[64:65, :].rearrange("p (a b) -> p a b", a=128)
                )
                rcp = spool.tile([128, 8], bf16, tag="rcp", name=f"rcp{h}{is_x}")
                with nc.allow_low_precision(reason="bf16 softmax denom recip within budget"):
                    nc.vector.reciprocal(rcp[:], dsc[:])
                rrow = spool.tile([1, NSH], bf16, tag="rrow", name=f"rrow{h}{is_x}")
                nc.sync.dma_start(rrow[:], rcp[:])
                rb = spool.tile([64, NSH], bf16, tag="rb", name=f"rb{h}{is_x}")
                nc.sync.dma_start(
                    rb[:],
                    rrow[0:1, :].rearrange("p (x n) -> p x n", x=1)
                    .to_broadcast([1, 64, NSH]),
                )
                oslice = outT[h // 2][p0:p0 + 64, :]
                if is_x:
                    nc.vector.tensor_mul(oslice, ut[0:64, :], rb[:])
                else:
                    tmp = spool.tile([128, NSH], bf16, tag="tmp", name=f"tmp{h}")
                    nc.vector.tensor_mul(tmp[p0:p0 + 64, :], ut[0:64, :], rb[:])
                    nc.vector.tensor_add(oslice, oslice, tmp[p0:p0 + 64, :])

            proj_dt(0)
            emitted_v = False
            for h in range(H):
                if h >= 2 and h % 2 == 0:
                    proj_dt(h // 2)
                p0 = 64 * (h % 2)
                for src_is_x in (True, False):
                    qh = (qT[h // 2] if src_is_x else aT)[
                        p0:p0 + 64, :] if src_is_x else aT[p0:p0 + 64, h // 2, :]
                    ets = sims_exps(h, qh)
                    if not emitted_v:
                        proj_v()
                        emitted_v = True
                    ut_norm(h, src_is_x, ets)

            # ---- probs_avg: single K=512 contraction = sum over heads ----
            for jt in range(JT):
                for ic in range(IC):
                    ps = pmix.tile([128, 512], f32, tag="mix")
                    for dt in range(QT):
                        nc.tensor.matmul(
                            ps[:],
                            kT[dt][:, jt * 128:(jt + 1) * 128],
                            qT[dt][:, ic * 512:(ic + 1) * 512],
                            start=(dt == 0), stop=(dt == QT - 1),
                        )
                    pst = stpool.tile([128, 512], f32, tag="probs")
                    nc.vector.tensor_scalar_mul(pst[:], ps[:], 1.0 / H)
                    nc.sync.dma_start(
                        probsT_d[jt * 128:(jt + 1) * 128, ic * 512:(ic + 1) * 512],
                        pst[:],
                    )

            # ---- stage 4: out = merged @ Wo + bo ----
            for it in range(IT):
                ps = pmix.tile([128, 512], f32, tag="mix")
                for dblk in range(QT):
                    nc.tensor.matmul(
                        ps[:],
                        outT[dblk][:, it * 128:(it + 1) * 128],
                        wo[:, dblk, :],
                        start=(dblk == 0), stop=(dblk == QT - 1),
                    )
                fin = stpool.tile([128, QD], f32, tag="fin")
                nc.vector.tensor_add(fin[:], ps[:], bob[:])
                nc.sync.dma_start(out_d[it * 128:(it + 1) * 128, :], fin[:])

    _split_waits(nc, mybir)
    return nc


def _split_waits(nc, mybir, max_waits=1):
    """This container's walrus rejects instructions with more than one sync
    wait; hoist excess waits onto same-engine NoOps placed just before (per-
    engine streams are in-order, so this is semantically identical)."""
    ctr = 0
    for bb in nc.m.functions[0].blocks:
        out, changed = [], False
        for inst in bb.instructions:
            si = inst.sync_info
            waits = list(si.on_wait) if si is not None and si.on_wait else []
            if len(waits) > max_waits:
                changed = True
                for w in waits[:-max_waits]:
                    ctr += 1
                    out.append(mybir.InstNoOp(
                        name=f"waitsplit-{ctr}",
                        sync_info=mybir.SyncInfo(on_wait=[w], on_update=[]),
                        bass_nofuse=True,
                        engine=inst.engine,
                    ))
                inst.sync_info = mybir.SyncInfo(
                    on_wait=waits[-max_waits:],
                    on_update=list(si.on_update) if si.on_update else [],
                )
            out.append(inst)
        if changed:
            bb.instructions = out


def kernel(x, context, adapt, Wq, Wk, Wv, Wo, bo):
    from concourse.bass_utils import run_bass_kernel_spmd

    if "nc" not in _cache:
        _cache["nc"] = _build()
    nc = _cache["nc"]

    x = np.asarray(x, dtype=np.float32)
    context = np.asarray(context, dtype=np.float32)
    adapt = np.asarray(adapt, dtype=np.float32)
    Wq = np.asarray(Wq, dtype=np.float32)
    Wk = np.asarray(Wk, dtype=np.float32)
    Wv = np.asarray(Wv, dtype=np.float32)
    Wo = np.asarray(Wo, dtype=np.float32)
    bo = np.asarray(bo, dtype=np.float32)

    bf = lambda a: np.ascontiguousarray(a).astype(ml_dtypes.bfloat16)
    wq_b, wk_b = bf(Wq), bf(Wk * SCALE)
    wv_b, wo_b = bf(Wv), bf(Wo)
    bob = np.ascontiguousarray(np.broadcast_to(bo, (128, QD))).astype(np.float32)
    ctxT = [bf(context[b].T) for b in range(B)]

    in_maps = []
    for c in range(NCORES):
        b, r0 = c // (NCORES // B), (c % (NCORES // B)) * NSH
        in_maps.append({
            "xT": bf(x[b, r0:r0 + NSH].T),
            "aT": bf(adapt[b, r0:r0 + NSH].T),
            "ctxT": ctxT[b],
            "wq": wq_b, "wk": wk_b, "wv": wv_b, "wo": wo_b,
            "bob": bob,
        })

    res = run_bass_kernel_spmd(
        nc, in_maps, core_ids=list(range(NCORES)), **_cache.get("run_kwargs", {})
    )
    _cache["last_result"] = res

    out = np.empty((B, N, QD), np.float32)
    probs = np.empty((B, N, M), np.float32)
    for c in range(NCORES):
        b, r0 = c // (NCORES // B), (c % (NCORES // B)) * NSH
        out[b, r0:r0 + NSH] = res.results[c]["out"]
        probs[b, r0:r0 + NSH] = res.results[c]["probsT"].T
    return out, probs


# revision 20
# speedup vs baseline: 1.2175x; 1.2175x over previous
"""Trainium2 Bass kernel for nn_CrossAttention_42365557408181.

Dual-query cross-attention: B=2, N=4096 (query rows), M=1024 (context rows),
H=8 heads, DH=64, QD=512, CD=768. Returns (out [B,N,512] f32,
probs_avg [B,N,1024] f32 = mean over heads of scaled raw logits).

Sharding: pure data-parallel over (B x N) -> 8 shards of 1024 query rows,
one per NeuronCore; k/v are recomputed per core for its batch (no
collectives). All device matmuls contract on the partition dim, so the host
pre-transposes activations (xT/aT/ctxT) and the attention is computed in a
"simT" [j, i] layout:

  qT = Wq^T-contract(xT)              [512(hd), 1024(i)]
  kT = (Wk*scale)^T-contract(ctxT)    [512(hd), 1024(j)]   (scale folded in)
  v  = ctxT^T-contract(Wv)            [1024(j), 512(hd)]
  per head/pass: simT = kT_h .T-contract. qT_h  -> exp (no max-sub; |sim|<~6)
  U = ET .T-contract. [v_h | ones]    [i, 65]  (col 64 = softmax denominator)
  out_h = U[:, :64] / U[:, 64]        summed over both query passes
  probsT = kT .T-contract. qT * (1/H) (single K=512 matmul = sum over heads)
  final  = merged_out @ Wo + bo       (PE-transpose of merged out feeds Wo)

Compute dtype bf16 (f32 PSUM accumulation), outputs f32.
"""

import os
import sys

for _p in ("/opt/trn_rl_repo", "/root/.axon_site/_ro/trn_rl_repo"):
    if os.path.isdir(_p) and _p not in sys.path:
        sys.path.insert(0, _p)

import numpy as np
import ml_dtypes

B, N, M = 2, 4096, 1024
QD, CD, H, DH = 512, 768, 8, 64
SCALE = DH ** -0.5
NCORES = 8
NSH = (B * N) // NCORES  # 1024 query rows per core

_cache = {}


def _build():
    import concourse.bass as bass
    import concourse.mybir as mybir
    import concourse.tile as tile

    f32 = mybir.dt.float32
    bf16 = mybir.dt.bfloat16
    AF = mybir.ActivationFunctionType

    nc = bass.Bass("TRN2")

    xT_d = nc.declare_dram_parameter("xT", [QD, NSH], bf16, isOutput=False)
    aT_d = nc.declare_dram_parameter("aT", [QD, NSH], bf16, isOutput=False)
    ctxT_d = nc.declare_dram_parameter("ctxT", [CD, M], bf16, isOutput=False)
    wq_d = nc.declare_dram_parameter("wq", [QD, QD], bf16, isOutput=False)
    wk_d = nc.declare_dram_parameter("wk", [CD, QD], bf16, isOutput=False)
    wv_d = nc.declare_dram_parameter("wv", [CD, QD], bf16, isOutput=False)
    wo_d = nc.declare_dram_parameter("wo", [QD, QD], bf16, isOutput=False)
    bob_d = nc.declare_dram_parameter("bob", [128, QD], f32, isOutput=False)
    out_d = nc.declare_dram_parameter("out", [NSH, QD], f32, isOutput=True)
    probsT_d = nc.declare_dram_parameter("probsT", [M, NSH], f32, isOutput=True)

    QT, CT = QD // 128, CD // 128          # 4, 6 k-tiles
    IT, JT = NSH // 128, M // 128          # 8, 8 row tiles
    IC = NSH // 512                        # 2 i-chunks of 512

    with tile.TileContext(nc) as tc:
        with (
            tc.tile_pool(name="w", bufs=1) as wpool,
            tc.tile_pool(name="act", bufs=1) as apool,
            tc.tile_pool(name="et", bufs=20) as etpool,
            tc.tile_pool(name="small", bufs=4) as spool,
            tc.tile_pool(name="stage", bufs=4) as stpool,
            tc.tile_pool(name="ut", bufs=4) as utpool,
            tc.tile_pool(name="psim", bufs=2, space="PSUM") as psim,
            tc.tile_pool(name="pmix", bufs=4, space="PSUM") as pmix,
        ):
            # ---- load inputs ----
            xT = apool.tile([128, QT, NSH], bf16, tag="xT")
            aT = apool.tile([128, QT, NSH], bf16, tag="aT")
            ctxT = apool.tile([128, CT, M], bf16, tag="ctxT")
            wq = wpool.tile([128, QT, QD], bf16, tag="wq")
            wk = wpool.tile([128, CT, QD], bf16, tag="wk")
            wv = wpool.tile([128, CT, QD], bf16, tag="wv")
            wo = wpool.tile([128, QT, QD], bf16, tag="wo")
            bob = wpool.tile([128, QD], f32, tag="bob")

            nc.sync.dma_start(ctxT[:], ctxT_d.ap().rearrange("(t p) j -> p t j", p=128))
            nc.sync.dma_start(wk[:], wk_d.ap().rearrange("(t p) d -> p t d", p=128))
            nc.sync.dma_start(wq[:], wq_d.ap().rearrange("(t p) d -> p t d", p=128))
            nc.sync.dma_start(xT[:], xT_d.ap().rearrange("(t p) i -> p t i", p=128))
            nc.sync.dma_start(wv[:], wv_d.ap().rearrange("(t p) d -> p t d", p=128))
            nc.sync.dma_start(aT[:], aT_d.ap().rearrange("(t p) i -> p t i", p=128))
            nc.sync.dma_start(wo[:], wo_d.ap().rearrange("(t p) d -> p t d", p=128))
            nc.sync.dma_start(bob[:], bob_d.ap())

            # ---- projections (emitted interleaved with the head loop so
            # ScalarE starts exp work ~7us in instead of after the whole
            # projection phase) ----
            qT = [apool.tile([128, NSH], bf16, tag=f"qT{dt}", name=f"qT{dt}") for dt in range(QT)]
            kT = [apool.tile([128, M], bf16, tag=f"kT{dt}", name=f"kT{dt}") for dt in range(QT)]
            vaug = [apool.tile([128, H * 65], bf16, tag=f"vaug{jt}", name=f"vaug{jt}") for jt in range(JT)]
            outT = [apool.tile([128, NSH], bf16, tag=f"outT{dt}", name=f"outT{dt}") for dt in range(QT)]

            def proj_dt(dt):
                for jc in range(M // 512):
                    ps = pmix.tile([128, 512], f32, tag="mix", name=f"psk{dt}{jc}")
                    for ct in range(CT):
                        nc.tensor.matmul(
                            ps[:],
                            wk[:, ct, dt * 128:(dt + 1) * 128],
                            ctxT[:, ct, jc * 512:(jc + 1) * 512],
                            start=(ct == 0), stop=(ct == CT - 1),
                        )
                    nc.vector.tensor_copy(kT[dt][:, jc * 512:(jc + 1) * 512], ps[:])
                for ic in range(IC):
                    ps = pmix.tile([128, 512], f32, tag="mix", name=f"psq{dt}{ic}")
                    for ct in range(QT):
                        nc.tensor.matmul(
                            ps[:],
                            wq[:, ct, dt * 128:(dt + 1) * 128],
                            xT[:, ct, ic * 512:(ic + 1) * 512],
                            start=(ct == 0), stop=(ct == QT - 1),
                        )
                    nc.vector.tensor_copy(qT[dt][:, ic * 512:(ic + 1) * 512], ps[:])

            def proj_v():
                for jt in range(JT):
                    nc.vector.memset(
                        vaug[jt].rearrange("p (h c) -> p h c", c=65)[:, :, 64:65], 1.0
                    )
                    ps = pmix.tile([128, 512], f32, tag="mix", name=f"psv{jt}")
                    for ct in range(CT):
                        nc.tensor.matmul(
                            ps[:],
                            ctxT[:, ct, jt * 128:(jt + 1) * 128],
                            wv[:, ct, :],
                            start=(ct == 0), stop=(ct == CT - 1),
                        )
                    nc.vector.tensor_copy(
                        vaug[jt].rearrange("p (h c) -> p h c", c=65)[:, :, 0:64],
                        ps[:].rearrange("p (h c) -> p h c", c=64),
                    )

            def sims_exps(h, qh):
                ets = []
                for jt in range(JT):
                    p0 = 64 * (h % 2)
                    kh = kT[h // 2][p0:p0 + 64, :]
                    ps = psim.tile([128, NSH], f32, tag="sim", name=f"sim{h}{jt}")
                    for ic in range(IC):
                        nc.tensor.matmul(
                            ps[:, ic * 512:(ic + 1) * 512],
                            kh[:, jt * 128:(jt + 1) * 128],
                            qh[:, ic * 512:(ic + 1) * 512],
                            start=True, stop=True,
                        )
                    et = etpool.tile([128, NSH], bf16, tag="et", name=f"et{h}{jt}")
                    nc.scalar.activation(et[:], ps[:], AF.Exp)
                    ets.append(et)
                return ets

            def ut_norm(h, is_x, ets):
                p0 = 64 * (h % 2)
                ut = utpool.tile([65, NSH], bf16, tag="ut", name=f"ut{h}{is_x}")
                for ic in range(IC):
                    psu = pmix.tile([65, 512], f32, tag="mix", name=f"psu{h}{ic}")
                    for jt in range(JT):
                        nc.tensor.matmul(
                            psu[:],
                            vaug[jt][:, h * 65:(h + 1) * 65],
                            ets[jt][:, ic * 512:(ic + 1) * 512],
                            start=(jt == 0), stop=(jt == JT - 1),
                        )
                    nc.vector.tensor_copy(ut[:, ic * 512:(ic + 1) * 512], psu[:])
                dsc = spool.tile([128, 8], bf16, tag="dsc", name=f"dsc{h}{is_x}")
                nc.sync.dma_start(
                    dsc[:], ut[64:65, :].rearrange("p (a b) -> p a b", a=128)
                )
                rcp = spool.tile([128, 8], bf16, tag="rcp", name=f"rcp{h}{is_x}")
                with nc.allow_low_precision(reason="bf16 softmax denom recip within budget"):
                    nc.vector.reciprocal(rcp[:], dsc[:])
                rrow = spool.tile([1, NSH], bf16, tag="rrow", name=f"rrow{h}{is_x}")
                nc.sync.dma_start(rrow[:], rcp[:])
                rb = spool.tile([64, NSH], bf16, tag="rb", name=f"rb{h}{is_x}")
                nc.sync.dma_start(
                    rb[:],
                    rrow[0:1, :].rearrange("p (x n) -> p x n", x=1)
                    .to_broadcast([1, 64, NSH]),
                )
                oslice = outT[h // 2][p0:p0 + 64, :]
                if is_x:
                    nc.vector.tensor_mul(oslice, ut[0:64, :], rb[:])
                else:
                    tmp = spool.tile([128, NSH], bf16, tag="tmp", name=f"tmp{h}")
                    nc.vector.tensor_mul(tmp[p0:p0 + 64, :], ut[0:64, :], rb[:])
                    nc.vector.tensor_add(oslice, oslice, tmp[p0:p0 + 64, :])

            # probs chains as fillers: one chain = 4 accumulating MMs
            def probs_chain(jt, ic):
                ps = pmix.tile([128, 512], f32, tag="mix", name=f"psp{jt}{ic}")
                for dt in range(QT):
                    nc.tensor.matmul(
                        ps[:],
                        kT[dt][:, jt * 128:(jt + 1) * 128],
                        qT[dt][:, ic * 512:(ic + 1) * 512],
                        start=(dt == 0), stop=(dt == QT - 1),
                    )
                pst = stpool.tile([128, 512], f32, tag="probs", name=f"pst{jt}{ic}")
                nc.vector.tensor_scalar_mul(pst[:], ps[:], 1.0 / H)
                nc.sync.dma_start(
                    probsT_d[jt * 128:(jt + 1) * 128, ic * 512:(ic + 1) * 512],
                    pst[:],
                )

            probs_left = [(jt, ic) for jt in range(JT) for ic in range(IC)]

            def filler(h, is_x):
                # fill PE while ScalarE chews the 8 exp tiles of this pass
                if h == 0 and is_x:
                    proj_v()
                elif h == 0:
                    proj_dt(1)
                elif h == 1 and is_x:
                    proj_dt(2)
                elif h == 1:
                    proj_dt(3)
                else:
                    for _ in range(3):
                        if probs_left:
                            probs_chain(*probs_left.pop(0))

            proj_dt(0)
            for h in range(H):
                p0 = 64 * (h % 2)
                for src_is_x in (True, False):
                    qh = qT[h // 2][p0:p0 + 64, :] if src_is_x \
                        else aT[p0:p0 + 64, h // 2, :]
                    ets = sims_exps(h, qh)
                    filler(h, src_is_x)
                    ut_norm(h, src_is_x, ets)
            while probs_left:
                probs_chain(*probs_left.pop(0))

            # ---- stage 4: out = merged @ Wo + bo ----
            for it in range(IT):
                ps = pmix.tile([128, 512], f32, tag="mix")
                for dblk in range(QT):
                    nc.tensor.matmul(
                        ps[:],
                        outT[dblk][:, it * 128:(it + 1) * 128],
                        wo[:, dblk, :],
                        start=(dblk == 0), stop=(dblk == QT - 1),
                    )
                fin = stpool.tile([128, QD], f32, tag="fin")
                nc.vector.tensor_add(fin[:], ps[:], bob[:])
                nc.sync.dma_start(out_d[it * 128:(it + 1) * 128, :], fin[:])

    _split_waits(nc, mybir)
    return nc


def _split_waits(nc, mybir, max_waits=1):
    """This container's walrus rejects instructions with more than one sync
    wait; hoist excess waits onto same-engine NoOps placed just before (per-
    engine streams are in-order, so this is semantically identical)."""
    ctr = 0
    for bb in nc.m.functions[0].blocks:
        out, changed = [], False
        for inst in bb.instructions:
            si = inst.sync_info
            waits = list(si.on_wait) if si is not None and si.on_wait else []
            if len(waits) > max_waits:
                changed = True
                for w in waits[:-max_waits]:
                    ctr += 1
                    out.append(mybir.InstNoOp(
                        name=f"waitsplit-{ctr}",
                        sync_info=mybir.SyncInfo(on_wait=[w], on_update=[]),
                        bass_nofuse=True,
                        engine=inst.engine,
                    ))
                inst.sync_info = mybir.SyncInfo(
                    on_wait=waits[-max_waits:],
                    on_update=list(si.on_update) if si.on_update else [],
                )
            out.append(inst)
        if changed:
            bb.instructions = out


def kernel(x, context, adapt, Wq, Wk, Wv, Wo, bo):
    from concourse.bass_utils import run_bass_kernel_spmd

    if "nc" not in _cache:
        _cache["nc"] = _build()
    nc = _cache["nc"]

    x = np.asarray(x, dtype=np.float32)
    context = np.asarray(context, dtype=np.float32)
    adapt = np.asarray(adapt, dtype=np.float32)
    Wq = np.asarray(Wq, dtype=np.float32)
    Wk = np.asarray(Wk, dtype=np.float32)
    Wv = np.asarray(Wv, dtype=np.float32)
    Wo = np.asarray(Wo, dtype=np.float32)
    bo = np.asarray(bo, dtype=np.float32)

    bf = lambda a: np.ascontiguousarray(a).astype(ml_dtypes.bfloat16)
    wq_b, wk_b = bf(Wq), bf(Wk * SCALE)
    wv_b, wo_b = bf(Wv), bf(Wo)
    bob = np.ascontiguousarray(np.broadcast_to(bo, (128, QD))).astype(np.float32)
    ctxT = [bf(context[b].T) for b in range(B)]

    in_maps = []
    for c in range(NCORES):
        b, r0 = c // (NCORES // B), (c % (NCORES // B)) * NSH
        in_maps.append({
            "xT": bf(x[b, r0:r0 + NSH].T),
            "aT": bf(adapt[b, r0:r0 + NSH].T),
            "ctxT": ctxT[b],
            "wq": wq_b, "wk": wk_b, "wv": wv_b, "wo": wo_b,
            "bob": bob,
        })

    res = run_bass_kernel_spmd(
        nc, in_maps, core_ids=list(range(NCORES)), **_cache.get("run_kwargs", {})
    )
    _cache["last_result"] = res

    out = np.empty((B, N, QD), np.float32)
    probs = np.empty((B, N, M), np.float32)
    for c in range(NCORES):
        b, r0 = c // (NCORES // B), (c % (NCORES // B)) * NSH
        out[b, r0:r0 + NSH] = res.results[c]["out"]
        probs[b, r0:r0 + NSH] = res.results[c]["probsT"].T
    return out, probs


# revision 21
# speedup vs baseline: 1.2451x; 1.0227x over previous
"""Trainium2 Bass kernel for nn_CrossAttention_42365557408181.

Dual-query cross-attention: B=2, N=4096 (query rows), M=1024 (context rows),
H=8 heads, DH=64, QD=512, CD=768. Returns (out [B,N,512] f32,
probs_avg [B,N,1024] f32 = mean over heads of scaled raw logits).

Sharding: pure data-parallel over (B x N) -> 8 shards of 1024 query rows,
one per NeuronCore; k/v are recomputed per core for its batch (no
collectives). All device matmuls contract on the partition dim, so the host
pre-transposes activations (xT/aT/ctxT) and the attention is computed in a
"simT" [j, i] layout:

  qT = Wq^T-contract(xT)              [512(hd), 1024(i)]
  kT = (Wk*scale)^T-contract(ctxT)    [512(hd), 1024(j)]   (scale folded in)
  v  = ctxT^T-contract(Wv)            [1024(j), 512(hd)]
  per head/pass: simT = kT_h .T-contract. qT_h  -> exp (no max-sub; |sim|<~6)
  U = ET .T-contract. [v_h | ones]    [i, 65]  (col 64 = softmax denominator)
  out_h = U[:, :64] / U[:, 64]        summed over both query passes
  probsT = kT .T-contract. qT * (1/H) (single K=512 matmul = sum over heads)
  final  = merged_out @ Wo + bo       (PE-transpose of merged out feeds Wo)

Compute dtype bf16 (f32 PSUM accumulation), outputs f32.
"""

import os
import sys

for _p in ("/opt/trn_rl_repo", "/root/.axon_site/_ro/trn_rl_repo"):
    if os.path.isdir(_p) and _p not in sys.path:
        sys.path.insert(0, _p)

import numpy as np
import ml_dtypes

B, N, M = 2, 4096, 1024
QD, CD, H, DH = 512, 768, 8, 64
SCALE = DH ** -0.5
NCORES = 8
NSH = (B * N) // NCORES  # 1024 query rows per core

_cache = {}


def _build():
    import concourse.bass as bass
    import concourse.mybir as mybir
    import concourse.tile as tile

    f32 = mybir.dt.float32
    bf16 = mybir.dt.bfloat16
    AF = mybir.ActivationFunctionType

    nc = bass.Bass("TRN2")

    xT_d = nc.declare_dram_parameter("xT", [QD, NSH], bf16, isOutput=False)
    aT_d = nc.declare_dram_parameter("aT", [QD, NSH], bf16, isOutput=False)
    ctxT_d = nc.declare_dram_parameter("ctxT", [CD, M], bf16, isOutput=False)
    wq_d = nc.declare_dram_parameter("wq", [QD, QD], bf16, isOutput=False)
    wk_d = nc.declare_dram_parameter("wk", [CD, QD], bf16, isOutput=False)
    wv_d = nc.declare_dram_parameter("wv", [CD, QD], bf16, isOutput=False)
    wo_d = nc.declare_dram_parameter("wo", [QD, QD], bf16, isOutput=False)
    bob_d = nc.declare_dram_parameter("bob", [128, QD], f32, isOutput=False)
    out_d = nc.declare_dram_parameter("out", [NSH, QD], f32, isOutput=True)
    probsT_d = nc.declare_dram_parameter("probsT", [M, NSH], f32, isOutput=True)

    QT, CT = QD // 128, CD // 128          # 4, 6 k-tiles
    IT, JT = NSH // 128, M // 128          # 8, 8 row tiles
    IC = NSH // 512                        # 2 i-chunks of 512

    with tile.TileContext(nc) as tc:
        with (
            tc.tile_pool(name="w", bufs=1) as wpool,
            tc.tile_pool(name="act", bufs=1) as apool,
            tc.tile_pool(name="et", bufs=20) as etpool,
            tc.tile_pool(name="small", bufs=4) as spool,
            tc.tile_pool(name="stage", bufs=4) as stpool,
            tc.tile_pool(name="ut", bufs=4) as utpool,
            tc.tile_pool(name="psim", bufs=2, space="PSUM") as psim,
            tc.tile_pool(name="pmix", bufs=4, space="PSUM") as pmix,
        ):
            # ---- load inputs ----
            xT = apool.tile([128, QT, NSH], bf16, tag="xT")
            aT = apool.tile([128, QT, NSH], bf16, tag="aT")
            ctxT = apool.tile([128, CT, M], bf16, tag="ctxT")
            wq = wpool.tile([128, QT, QD], bf16, tag="wq")
            wk = wpool.tile([128, CT, QD], bf16, tag="wk")
            wv = wpool.tile([128, CT, QD], bf16, tag="wv")
            wo = wpool.tile([128, QT, QD], bf16, tag="wo")
            bob = wpool.tile([128, QD], f32, tag="bob")

            nc.sync.dma_start(ctxT[:], ctxT_d.ap().rearrange("(t p) j -> p t j", p=128))
            nc.sync.dma_start(wk[:], wk_d.ap().rearrange("(t p) d -> p t d", p=128))
            nc.sync.dma_start(wq[:], wq_d.ap().rearrange("(t p) d -> p t d", p=128))
            nc.sync.dma_start(xT[:], xT_d.ap().rearrange("(t p) i -> p t i", p=128))
            nc.sync.dma_start(wv[:], wv_d.ap().rearrange("(t p) d -> p t d", p=128))
            nc.sync.dma_start(aT[:], aT_d.ap().rearrange("(t p) i -> p t i", p=128))
            nc.sync.dma_start(wo[:], wo_d.ap().rearrange("(t p) d -> p t d", p=128))
            nc.sync.dma_start(bob[:], bob_d.ap())

            # ---- projections (emitted interleaved with the head loop so
            # ScalarE starts exp work ~7us in instead of after the whole
            # projection phase) ----
            qT = [apool.tile([128, NSH], bf16, tag=f"qT{dt}", name=f"qT{dt}") for dt in range(QT)]
            kT = [apool.tile([128, M], bf16, tag=f"kT{dt}", name=f"kT{dt}") for dt in range(QT)]
            vaug = [apool.tile([128, H * 65], bf16, tag=f"vaug{jt}", name=f"vaug{jt}") for jt in range(JT)]
            outT = [apool.tile([128, NSH], bf16, tag=f"outT{dt}", name=f"outT{dt}") for dt in range(QT)]

            def proj_dt(dt):
                for jc in range(M // 512):
                    ps = pmix.tile([128, 512], f32, tag="mix", name=f"psk{dt}{jc}")
                    for ct in range(CT):
                        nc.tensor.matmul(
                            ps[:],
                            wk[:, ct, dt * 128:(dt + 1) * 128],
                            ctxT[:, ct, jc * 512:(jc + 1) * 512],
                            start=(ct == 0), stop=(ct == CT - 1),
                        )
                    nc.vector.tensor_copy(kT[dt][:, jc * 512:(jc + 1) * 512], ps[:])
                for ic in range(IC):
                    ps = pmix.tile([128, 512], f32, tag="mix", name=f"psq{dt}{ic}")
                    for ct in range(QT):
                        nc.tensor.matmul(
                            ps[:],
                            wq[:, ct, dt * 128:(dt + 1) * 128],
                            xT[:, ct, ic * 512:(ic + 1) * 512],
                            start=(ct == 0), stop=(ct == QT - 1),
                        )
                    nc.vector.tensor_copy(qT[dt][:, ic * 512:(ic + 1) * 512], ps[:])

            def proj_v(j0, j1):
                for jt in range(j0, j1):
                    nc.vector.memset(
                        vaug[jt].rearrange("p (h c) -> p h c", c=65)[:, :, 64:65], 1.0
                    )
                    ps = pmix.tile([128, 512], f32, tag="mix", name=f"psv{jt}")
                    for ct in range(CT):
                        nc.tensor.matmul(
                            ps[:],
                            ctxT[:, ct, jt * 128:(jt + 1) * 128],
                            wv[:, ct, :],
                            start=(ct == 0), stop=(ct == CT - 1),
                        )
                    nc.vector.tensor_copy(
                        vaug[jt].rearrange("p (h c) -> p h c", c=65)[:, :, 0:64],
                        ps[:].rearrange("p (h c) -> p h c", c=64),
                    )

            def sim_exp_jt(h, qh, jt):
                p0 = 64 * (h % 2)
                kh = kT[h // 2][p0:p0 + 64, :]
                ps = psim.tile([128, NSH], f32, tag="sim", name=f"sim{h}{jt}")
                for ic in range(IC):
                    nc.tensor.matmul(
                        ps[:, ic * 512:(ic + 1) * 512],
                        kh[:, jt * 128:(jt + 1) * 128],
                        qh[:, ic * 512:(ic + 1) * 512],
                        start=True, stop=True,
                    )
                et = etpool.tile([128, NSH], bf16, tag="et", name=f"et{h}{jt}")
                nc.scalar.activation(et[:], ps[:], AF.Exp)
                return et

            def norm_chain(h, is_x, ut):
                p0 = 64 * (h % 2)
                dsc = spool.tile([128, 8], bf16, tag="dsc", name=f"dsc{h}{is_x}")
                nc.sync.dma_start(
                    dsc[:], ut[64:65, :].rearrange("p (a b) -> p a b", a=128)
                )
                rcp = spool.tile([128, 8], bf16, tag="rcp", name=f"rcp{h}{is_x}")
                with nc.allow_low_precision(reason="bf16 softmax denom recip within budget"):
                    nc.vector.reciprocal(rcp[:], dsc[:])
                rrow = spool.tile([1, NSH], bf16, tag="rrow", name=f"rrow{h}{is_x}")
                nc.sync.dma_start(rrow[:], rcp[:])
                rb = spool.tile([64, NSH], bf16, tag="rb", name=f"rb{h}{is_x}")
                nc.sync.dma_start(
                    rb[:],
                    rrow[0:1, :].rearrange("p (x n) -> p x n", x=1)
                    .to_broadcast([1, 64, NSH]),
                )
                oslice = outT[h // 2][p0:p0 + 64, :]
                if is_x:
                    nc.vector.tensor_mul(oslice, ut[0:64, :], rb[:])
                else:
                    tmp = spool.tile([128, NSH], bf16, tag="tmp", name=f"tmp{h}")
                    nc.vector.tensor_mul(tmp[p0:p0 + 64, :], ut[0:64, :], rb[:])
                    nc.vector.tensor_add(oslice, oslice, tmp[p0:p0 + 64, :])

            # probs chains as fillers: one chain = 4 accumulating MMs
            def probs_chain(jt, ic):
                ps = pmix.tile([128, 512], f32, tag="mix", name=f"psp{jt}{ic}")
                for dt in range(QT):
                    nc.tensor.matmul(
                        ps[:],
                        kT[dt][:, jt * 128:(jt + 1) * 128],
                        qT[dt][:, ic * 512:(ic + 1) * 512],
                        start=(dt == 0), stop=(dt == QT - 1),
                    )
                pst = stpool.tile([128, 512], f32, tag="probs", name=f"pst{jt}{ic}")
                nc.vector.tensor_scalar_mul(pst[:], ps[:], 1.0 / H)
                nc.sync.dma_start(
                    probsT_d[jt * 128:(jt + 1) * 128, ic * 512:(ic + 1) * 512],
                    pst[:],
                )

            probs_left = [(jt, ic) for jt in range(JT) for ic in range(IC)]

            def filler(h, is_x):
                if h == 0 and is_x:
                    proj_v(4, JT)      # v[4..7]
                elif h == 0:
                    proj_dt(1)
                elif h == 1 and is_x:
                    proj_dt(2)
                elif h == 1:
                    proj_dt(3)
                else:
                    if probs_left:
                        probs_chain(*probs_left.pop(0))

            # software pipeline: period (h, pass); the PREVIOUS period's UT
            # matmuls interleave between this period's sim+exp pairs so the
            # PE tape never stalls on the tail exp of a period.
            periods = [(h, is_x) for h in range(H) for is_x in (True, False)]
            proj_dt(0)
            proj_v(0, 4)               # v[0..3] in the prelude
            prev = None
            for h, is_x in periods:
                p0 = 64 * (h % 2)
                qh = qT[h // 2][p0:p0 + 64, :] if is_x \
                    else aT[p0:p0 + 64, h // 2, :]
                if prev is not None:
                    ph, pis_x, pets = prev
                    psu = [pmix.tile([65, 512], f32, tag="mix", name=f"psu{ph}{pis_x}{ic}")
                           for ic in range(IC)]
                ets = []
                for jt in range(JT):
                    ets.append(sim_exp_jt(h, qh, jt))
                    if prev is not None:
                        for ic in range(IC):
                            nc.tensor.matmul(
                                psu[ic][:],
                                vaug[jt][:, ph * 65:(ph + 1) * 65],
                                pets[jt][:, ic * 512:(ic + 1) * 512],
                                start=(jt == 0), stop=(jt == JT - 1),
                            )
                if prev is not None:
                    ut = utpool.tile([65, NSH], bf16, tag="ut", name=f"ut{ph}{pis_x}")
                    for ic in range(IC):
                        nc.vector.tensor_copy(ut[:, ic * 512:(ic + 1) * 512], psu[ic][:])
                    norm_chain(ph, pis_x, ut)
                filler(h, is_x)
                prev = (h, is_x, ets)

            # drain the last period's UT + norm
            ph, pis_x, pets = prev
            psu = [pmix.tile([65, 512], f32, tag="mix", name=f"psuZ{ic}")
                   for ic in range(IC)]
            for jt in range(JT):
                for ic in range(IC):
                    nc.tensor.matmul(
                        psu[ic][:],
                        vaug[jt][:, ph * 65:(ph + 1) * 65],
                        pets[jt][:, ic * 512:(ic + 1) * 512],
                        start=(jt == 0), stop=(jt == JT - 1),
                    )
            ut = utpool.tile([65, NSH], bf16, tag="ut", name="utZ")
            for ic in range(IC):
                nc.vector.tensor_copy(ut[:, ic * 512:(ic + 1) * 512], psu[ic][:])
            norm_chain(ph, pis_x, ut)
            while probs_left:
                probs_chain(*probs_left.pop(0))

            # ---- stage 4: out = merged @ Wo + bo ----
            for it in range(IT):
                ps = pmix.tile([128, 512], f32, tag="mix")
                for dblk in range(QT):
                    nc.tensor.matmul(
                        ps[:],
                        outT[dblk][:, it * 128:(it + 1) * 128],
                        wo[:, dblk, :],
                        start=(dblk == 0), stop=(dblk == QT - 1),
                    )
                fin = stpool.tile([128, QD], f32, tag="fin")
                nc.vector.tensor_add(fin[:], ps[:], bob[:])
                nc.sync.dma_start(out_d[it * 128:(it + 1) * 128, :], fin[:])

    _split_waits(nc, mybir)
    return nc


def _split_waits(nc, mybir, max_waits=1):
    """This container's walrus rejects instructions with more than one sync
    wait; hoist excess waits onto same-engine NoOps placed just before (per-
    engine streams are in-order, so this is semantically identical)."""
    ctr = 0
    for bb in nc.m.functions[0].blocks:
        out, changed = [], False
        for inst in bb.instructions:
            si = inst.sync_info
            waits = list(si.on_wait) if si is not None and si.on_wait else []
            if len(waits) > max_waits:
                changed = True
                for w in waits[:-max_waits]:
                    ctr += 1
                    out.append(mybir.InstNoOp(
                        name=f"waitsplit-{ctr}",
                        sync_info=mybir.SyncInfo(on_wait=[w], on_update=[]),
                        bass_nofuse=True,
                        engine=inst.engine,
                    ))
                inst.sync_info = mybir.SyncInfo(
                    on_wait=waits[-max_waits:],
                    on_update=list(si.on_update) if si.on_update else [],
                )
            out.append(inst)
        if changed:
            bb.instructions = out


def kernel(x, context, adapt, Wq, Wk, Wv, Wo, bo):
    from concourse.bass_utils import run_bass_kernel_spmd

    if "nc" not in _cache:
        _cache["nc"] = _build()
    nc = _cache["nc"]

    x = np.asarray(x, dtype=np.float32)
    context = np.asarray(context, dtype=np.float32)
    adapt = np.asarray(adapt, dtype=np.float32)
    Wq = np.asarray(Wq, dtype=np.float32)
    Wk = np.asarray(Wk, dtype=np.float32)
    Wv = np.asarray(Wv, dtype=np.float32)
    Wo = np.asarray(Wo, dtype=np.float32)
    bo = np.asarray(bo, dtype=np.float32)

    bf = lambda a: np.ascontiguousarray(a).astype(ml_dtypes.bfloat16)
    wq_b, wk_b = bf(Wq), bf(Wk * SCALE)
    wv_b, wo_b = bf(Wv), bf(Wo)
    bob = np.ascontiguousarray(np.broadcast_to(bo, (128, QD))).astype(np.float32)
    ctxT = [bf(context[b].T) for b in range(B)]

    in_maps = []
    for c in range(NCORES):
        b, r0 = c // (NCORES // B), (c % (NCORES // B)) * NSH
        in_maps.append({
            "xT": bf(x[b, r0:r0 + NSH].T),
            "aT": bf(adapt[b, r0:r0 + NSH].T),
            "ctxT": ctxT[b],
            "wq": wq_b, "wk": wk_b, "wv": wv_b, "wo": wo_b,
            "bob": bob,
        })

    res = run_bass_kernel_spmd(
        nc, in_maps, core_ids=list(range(NCORES)), **_cache.get("run_kwargs", {})
    )
    _cache["last_result"] = res

    out = np.empty((B, N, QD), np.float32)
    probs = np.empty((B, N, M), np.float32)
    for c in range(NCORES):
        b, r0 = c // (NCORES // B), (c % (NCORES // B)) * NSH
        out[b, r0:r0 + NSH] = res.results[c]["out"]
        probs[b, r0:r0 + NSH] = res.results[c]["probsT"].T
    return out, probs
